# revision 14
# baseline (speedup 1.0000x reference)
"""BoundaryAwareLoss Trainium2 kernel.

Sharding: 8 (batch, instance-channel) pairs -> 8 cores, one 128^3 volume each.
Per-core layout: partition dim = D (128), free dim = H*W (16384), bf16 on wire
(targets/spatial_mask are {0,1} so bf16 is lossless; logits bf16 costs ~1e-4 rel).

Erosion (6-connected, border=0) per iteration, per 512-col chunk:
  psum = Tri_d @ m            (d-axis 3-sum via tridiagonal matmul over partitions)
       + I @ m[f-1] + I @ m[f+1]   (w+-1 shifts, PE identity matmuls, PSUM-accumulated)
       + I @ (m[f-128] + m[f+128]) (h+-1 pair-sum on DVE, aligned -> 2x mode)
  eroded = relu(psum - 6)     (sum==7 -> 1, else 0; exact: integer sums)
  then w-edge columns (w=0, w=127) forced to 0.
Halo: tiles padded with 128 zeros each side -> h-edges read correct zeros;
d-edges handled by the tridiagonal matrix rows; w-edges forced after threshold.

BCE: bce = softplus((1-2T)*L) (exact identity for the stable BCE-with-logits).
Partials: r = bce*SM, q = r*(T-E2); reduced over partitions with ones-vector
matmuls accumulated into PSUM; per-column partials DMA'd out.
Host: loss = sum_i m_i*(sum r_i + 4*sum q_i) / max(sum_i m_i*sum SM_i, 1).
"""

import os
import sys

import numpy as np

INSTANCE_INDICES = (1, 3, 5, 7)
D = 128
V = 128 * 128  # free elements per partition
PAD = 128
CH = 2048  # DVE chunk
MM = 512   # matmul / psum chunk
NCH = V // CH
NMM_PER_CH = CH // MM


def _ensure_concourse():
    for p in ("/opt/trn_rl_repo", "/root/.axon_site/_ro/trn_rl_repo"):
        if os.path.isdir(p) and p not in sys.path:
            sys.path.insert(0, p)


_NC_CACHE = {}


def _build_nc(variant="expln"):
    # "expln" composes softplus as Ln(1+Exp(z)): the compiler's act tables
    # have no 'softplus' entry (lower_act fails), and CoreSim lacks it too.
    # exp/ln/relu/copy all live in one act func set -> no table reloads.
    if variant in _NC_CACHE:
        return _NC_CACHE[variant]
    _ensure_concourse()
    import concourse.bacc as bacc
    import concourse.mybir as mybir
    from concourse.alu_op_type import AluOpType
    from concourse.tile import TileContext

    AF = mybir.ActivationFunctionType
    bf16 = mybir.dt.bfloat16
    f32 = mybir.dt.float32

    nc = bacc.Bacc(trn_type="TRN2")
    Ldr = nc.dram_tensor("lg", [D, V], bf16, kind="ExternalInput")
    Tdr = nc.dram_tensor("tg", [D, V], bf16, kind="ExternalInput")
    Sdr = nc.dram_tensor("sm", [D, V], bf16, kind="ExternalInput")
    Cdr = nc.dram_tensor("cst", [D, 256], bf16, kind="ExternalInput")
    Odr = nc.dram_tensor("out", [1, 3 * MM], f32, kind="ExternalOutput")

    with TileContext(nc) as tc:
        with (
            tc.tile_pool(name="persist", bufs=1) as pp,
            tc.tile_pool(name="stream", bufs=2) as sp,
            tc.tile_pool(name="temps", bufs=2) as tp,
            tc.tile_pool(name="a2pool", bufs=4) as a2p,
            tc.tile_pool(name="epsum", bufs=4, space="PSUM") as psp,
            tc.tile_pool(name="accpsum", bufs=1, space="PSUM") as pacc,
        ):
            consts = pp.tile([D, 256], bf16)
            nc.sync.dma_start(consts[:], Cdr[:])
            tri = consts[:, 0:128]
            idm = consts[:, 128:256]
            ones = pp.tile([D, 1], bf16)
            nc.gpsimd.memset(ones[:], 1.0)
            neg6 = pp.tile([D, 1], f32)
            nc.gpsimd.memset(neg6[:], -6.0)

            Tt = pp.tile([D, PAD + V + PAD], bf16)
            E1 = pp.tile([D, PAD + V + PAD], bf16)
            E2 = pp.tile([D, PAD + V + PAD], bf16)
            for t in (Tt, E1, E2):
                # all memsets pinned to gpsimd: one vector-clock proc for
                # consumers to wait on (codegen has few sync-wait slots)
                nc.gpsimd.memset(t[:, 0:PAD], 0.0)
                nc.gpsimd.memset(t[:, PAD + V:], 0.0)
            for g in range(NCH):
                nc.sync.dma_start(
                    Tt[:, PAD + g * CH: PAD + (g + 1) * CH],
                    Tdr[:, g * CH: (g + 1) * CH],
                )

            def erode(src, dst):
                for g in range(NCH):
                    F0 = PAD + g * CH
                    for j in range(NMM_PER_CH):
                        f0 = F0 + j * MM
                        a2 = a2p.tile([D, MM], bf16, tag="a2", name="a2")
                        nc.vector.tensor_tensor(
                            a2[:],
                            src[:, f0 - 128: f0 - 128 + MM],
                            src[:, f0 + 128: f0 + 128 + MM],
                            AluOpType.add,
                        )
                        ps = psp.tile([D, MM], f32, tag="eps", name="ps")
                        nc.tensor.matmul(ps[:], tri, src[:, f0:f0 + MM],
                                         start=True, stop=False)
                        nc.tensor.matmul(ps[:], idm, src[:, f0 - 1:f0 - 1 + MM],
                                         start=False, stop=False)
                        nc.tensor.matmul(ps[:], idm, src[:, f0 + 1:f0 + 1 + MM],
                                         start=False, stop=False)
                        nc.tensor.matmul(ps[:], idm, a2[:],
                                         start=False, stop=True)
                        nc.scalar.activation(dst[:, f0:f0 + MM], ps[:],
                                             AF.Relu, bias=neg6[:])
                    edge = dst[:, F0:F0 + CH].rearrange("p (h w) -> p h w", w=128)
                    nc.gpsimd.memset(edge[:, :, 0:1], 0.0)
                    nc.gpsimd.memset(edge[:, :, 127:128], 0.0)

            erode(Tt, E1)
            erode(E1, E2)

            ps_r = pacc.tile([D, MM], f32, tag="psr")
            ps_q = pacc.tile([D, MM], f32, tag="psq")
            ps_s = pacc.tile([D, MM], f32, tag="pss")
            for g in range(NCH):
                F0n = g * CH
                F0 = PAD + g * CH
                Lt = sp.tile([D, CH], bf16, tag="lt", name="Lt")
                nc.sync.dma_start(Lt[:], Ldr[:, F0n:F0n + CH])
                St = sp.tile([D, CH], bf16, tag="st", name="St")
                nc.sync.dma_start(St[:], Sdr[:, F0n:F0n + CH])
                s_ = tp.tile([D, CH], bf16, tag="s", name="s_")
                nc.vector.tensor_scalar(s_[:], Tt[:, F0:F0 + CH], -2.0, 1.0,
                                        AluOpType.mult, AluOpType.add)
                z = tp.tile([D, CH], bf16, tag="z", name="z")
                nc.vector.tensor_tensor(z[:], Lt[:], s_[:], AluOpType.mult)
                bce = tp.tile([D, CH], bf16, tag="bce", name="bce")
                if variant == "native":
                    nc.scalar.activation(bce[:], z[:], AF.Softplus)
                else:
                    ez = tp.tile([D, CH], bf16, tag="ez", name="ez")
                    nc.scalar.activation(ez[:], z[:], AF.Exp)
                    nc.scalar.activation(bce[:], ez[:], AF.Ln, bias=1.0)
                r = tp.tile([D, CH], bf16, tag="r", name="r")
                nc.vector.tensor_tensor(r[:], bce[:], St[:], AluOpType.mult)
                u = tp.tile([D, CH], bf16, tag="u", name="u")
                nc.vector.tensor_tensor(u[:], Tt[:, F0:F0 + CH],
                                        E2[:, F0:F0 + CH], AluOpType.subtract)
                q = tp.tile([D, CH], bf16, tag="q", name="q")
                nc.vector.tensor_tensor(q[:], r[:], u[:], AluOpType.mult)
                for j in range(NMM_PER_CH):
                    sl = slice(j * MM, (j + 1) * MM)
                    first = g == 0 and j == 0
                    last = g == NCH - 1 and j == NMM_PER_CH - 1
                    nc.tensor.matmul(ps_r[:1], ones[:], r[:, sl], start=first,
                                     stop=last, skip_group_check=True)
                    nc.tensor.matmul(ps_q[:1], ones[:], q[:, sl], start=first,
                                     stop=last, skip_group_check=True)
                    nc.tensor.matmul(ps_s[:1], ones[:], St[:, sl], start=first,
                                     stop=last, skip_group_check=True)

            outsb = pp.tile([1, 3 * MM], f32)
            nc.any.tensor_copy(outsb[:, 0:MM], ps_r[:1])
            nc.any.tensor_copy(outsb[:, MM:2 * MM], ps_q[:1])
            nc.any.tensor_copy(outsb[:, 2 * MM:3 * MM], ps_s[:1])
            nc.sync.dma_start(Odr[:], outsb[:])

    nc.compile()  # bacc lowering: event-semaphore legalization, reg alloc
    _NC_CACHE[variant] = nc
    return nc


def _consts_np():
    import ml_dtypes
    tri = (np.eye(128) + np.eye(128, k=1) + np.eye(128, k=-1))
    idm = np.eye(128)
    return np.concatenate([tri, idm], axis=1).astype(ml_dtypes.bfloat16)


def make_in_maps(logits, targets, spatial_mask):
    import ml_dtypes
    bf16 = ml_dtypes.bfloat16
    cst = _consts_np()
    sm_b = [
        np.ascontiguousarray(spatial_mask[b, 0].reshape(D, V)).astype(bf16)
        for b in range(2)
    ]
    in_maps = []
    for i in range(8):
        b, k = divmod(i, 4)
        ch = INSTANCE_INDICES[k]
        in_maps.append({
            "lg": np.ascontiguousarray(logits[b, ch].reshape(D, V)).astype(bf16),
            "tg": np.ascontiguousarray(targets[b, ch].reshape(D, V)).astype(bf16),
            "sm": sm_b[b],
            "cst": cst,
        })
    return in_maps


LAST_RESULTS = None  # set by kernel(); test.py reads exec_time_ns from it


def _combine(mask, per_core_outs):
    total = 0.0
    nvox = 0.0
    for i, o in enumerate(per_core_outs):
        b, k = divmod(i, 4)
        m = float(np.asarray(mask)[b, INSTANCE_INDICES[k]])
        o = o.astype(np.float64)
        total += m * (o[0, :MM].sum() + 4.0 * o[0, MM:2 * MM].sum())
        nvox += m * o[0, 2 * MM:3 * MM].sum()
    val = total / max(nvox, 1.0) if nvox > 0 else 0.0
    return np.float32(val)


def kernel(logits, targets, mask, spatial_mask):
    global LAST_RESULTS
    _ensure_concourse()
    from concourse import bass_utils

    nc = _build_nc()
    in_maps = make_in_maps(logits, targets, spatial_mask)
    res = bass_utils.run_bass_kernel_spmd(
        nc, in_maps, core_ids=list(range(8)), trace=False,
    )
    LAST_RESULTS = res
    return _combine(mask, [r["out"] for r in res.results])


def bench(logits, targets, mask, spatial_mask, n_iters=16):
    """Run via PJRT with device-resident inputs; time steady-state execs.

    Returns (value, per_exec_seconds). Mirrors bass2jax.run_bass_via_pjrt's
    multi-core path but keeps inputs on device so repeat calls measure the
    NEFF execution (incl. its HBM->SBUF DMAs) rather than host transfers.
    """
    _ensure_concourse()
    import time

    import jax
    import concourse.mybir as mybir
    from concourse import bass2jax
    from jax.sharding import Mesh, NamedSharding, PartitionSpec
    from jax.experimental.shard_map import shard_map

    nc = _build_nc()
    in_maps = make_in_maps(logits, targets, spatial_mask)
    n_cores = 8
    bass2jax.install_neuronx_cc_hook()

    partition_name = (nc.partition_id_tensor.name
                      if nc.partition_id_tensor else None)
    in_names, out_names, out_avals, zero_outs = [], [], [], []
    for alloc in nc.m.functions[0].allocations:
        if not isinstance(alloc, mybir.MemoryLocationSet):
            continue
        name = alloc.memorylocations[0].name
        if alloc.kind == "ExternalInput":
            if name != partition_name:
                in_names.append(name)
        elif alloc.kind == "ExternalOutput":
            out_names.append(name)
            shape = tuple(alloc.tensor_shape)
            dtype = mybir.dt.np(alloc.dtype)
            out_avals.append(jax.core.ShapedArray(shape, dtype))
            zero_outs.append(np.zeros(shape, dtype))
    n_params = len(in_names)
    n_outs = len(out_avals)
    all_in_names = list(in_names) + out_names
    if partition_name is not None:
        all_in_names.append(partition_name)
    donate = tuple(range(n_params, n_params + n_outs))

    def _body(*args):
        operands = list(args)
        if partition_name is not None:
            operands.append(bass2jax.partition_id_tensor())
        outs = bass2jax._bass_exec_p.bind(
            *operands,
            out_avals=tuple(out_avals),
            in_names=tuple(all_in_names),
            out_names=tuple(out_names),
            lowering_input_output_aliases=(),
            sim_require_finite=True,
            sim_require_nnan=True,
            nc=nc,
        )
        return tuple(outs)

    devices = jax.devices()[:n_cores]
    mesh = Mesh(np.asarray(devices), ("core",))
    in_specs = (PartitionSpec("core"),) * (n_params + n_outs)
    out_specs = (PartitionSpec("core"),) * len(out_names)
    sharded = jax.jit(
        shard_map(_body, mesh=mesh, in_specs=in_specs, out_specs=out_specs,
                  check_rep=False),
        donate_argnums=donate, keep_unused=True,
    )
    per_core = [[np.asarray(m[name]) for name in in_names] for m in in_maps]
    sh = NamedSharding(mesh, PartitionSpec("core"))
    dev_in = [
        jax.device_put(
            np.concatenate([per_core[c][i] for c in range(n_cores)], axis=0), sh)
        for i in range(n_params)
    ]
    def zeros():
        return [np.zeros((n_cores * z.shape[0], *z.shape[1:]), z.dtype)
                for z in zero_outs]

    out = sharded(*dev_in, *zeros())  # compile + correctness
    jax.block_until_ready(out)
    vals = [
        np.asarray(out[i]).reshape(n_cores, *out_avals[i].shape)
        for i in range(n_outs)
    ]
    value = _combine(mask, [vals[0][c] for c in range(n_cores)])

    # steady-state timing: enqueue n_iters executions, block once
    t0 = time.perf_counter()
    outs = []
    for _ in range(n_iters):
        outs.append(sharded(*dev_in, *zeros()))
    jax.block_until_ready(outs)
    dt = (time.perf_counter() - t0) / n_iters
    # single-call latency for comparison
    t0 = time.perf_counter()
    jax.block_until_ready(sharded(*dev_in, *zeros()))
    dt1 = time.perf_counter() - t0
    return value, dt, dt1
